# Initial kernel scaffold
#
"""Trainium2 Bass kernel for nn_Apply2DTform: batched affine image warp with
round-nearest bilinear sampling (grid_sample style).

Strategy (v1):
  - Pure data parallel: batch 32 -> 8 cores x 4 images.
  - Per image: build zero-padded image [513,514] in DRAM scratch.
  - Per 128-row output block: compute source coords x,y (exact fp order of the
    jax reference), x0/x1/y0/y1 + clips + bilinear weights on DVE/Act.
  - Gather the 4 taps with gpsimd indirect DMA: 2 descriptors/pixel, each
    fetching the (y0c, y0c+1) f32 pair of one source row.  The clipped-column
    case (y1c == y0c) is handled by folding W01 into W00 (pair's odd element
    then has zero weight).
  - Blend on DVE, DMA out.

kernel(**inputs) takes FULL inputs, returns FULL output (32,512,512,1) f32.
"""
import os
import sys

sys.path.insert(0, "/opt/trn_rl_repo")

import numpy as np

import concourse.bass as bass
import concourse.mybir as mybir
from concourse.bass_utils import run_bass_kernel_spmd

f32 = mybir.dt.float32
i32 = mybir.dt.int32

N_CORES = 8
B_PER = 4            # images per core
H = W = 512
PH, PW = 513, 514    # padded image rows x cols (row 512 zero, cols 512/513 zero)
NP_IMG = PH * PW     # padded image elements
MAGIC = 12582912.0   # 2^23 + 2^22 : add/sub rounds f32 to nearest-even integer

LAST_EXEC_NS = None


def _ax_host():
    # jnp.linspace(-1, 1, 512) in f32; jax computes iota*delta + start in f32
    i = np.arange(512, dtype=np.float32)
    delta = np.float32(2.0) / np.float32(511.0)
    return (i * delta + np.float32(-1.0)).astype(np.float32)


def _build_nc():
    nc = bass.Bass()
    img4 = nc.declare_dram_parameter("img4", [B_PER, H, W], f32, isOutput=False)
    tf4 = nc.declare_dram_parameter("tf4", [B_PER, 6], f32, isOutput=False)
    axg = nc.declare_dram_parameter("axg", [128, 4], f32, isOutput=False)
    ayg = nc.declare_dram_parameter("ayg", [1, W], f32, isOutput=False)
    out4 = nc.declare_dram_parameter("out4", [B_PER, H, W], f32, isOutput=True)
    imgp = nc.dram_tensor("imgp", [B_PER * NP_IMG], f32)

    ctx = []

    def sb(name, shape, dt=f32):
        t = nc.sbuf_tensor(name, shape, dt)
        ctx.append(t)
        return t.__enter__()

    s_dma = nc.semaphore("s_dma").__enter__()
    s_c = nc.semaphore("s_c").__enter__()

    z = sb("z", [128, PW])
    tc = sb("tc", [128, 6])
    axs = sb("axs", [128, 4])
    ays = sb("ays", [128, W])
    # math tiles
    x_t = sb("x_t", [128, W])
    y_t = sb("y_t", [128, W])
    u_t = sb("u_t", [128, W])
    x0 = sb("x0", [128, W])
    y0 = sb("y0", [128, W])
    x0c = sb("x0c", [128, W])
    x1c = sb("x1c", [128, W])
    y0c = sb("y0c", [128, W])
    y1c = sb("y1c", [128, W])
    fx1 = sb("fx1", [128, W])
    fx0 = sb("fx0", [128, W])
    fy1 = sb("fy1", [128, W])
    fy0 = sb("fy0", [128, W])
    w00 = sb("w00", [128, W])
    w01 = sb("w01", [128, W])
    w10 = sb("w10", [128, W])
    w11 = sb("w11", [128, W])
    eqt = sb("eqt", [128, W])
    offa = sb("offa", [128, W], i32)
    offb = sb("offb", [128, W], i32)
    ga = sb("ga", [128, 2 * W])
    gb = sb("gb", [128, 2 * W])
    acc = sb("acc", [128, W])
    tmp = sb("tmp", [128, W])
    # tiny per-image consts [128,1]
    m00 = sb("m00", [128, 1])
    m01 = sb("m01", [128, 1])
    m10 = sb("m10", [128, 1])
    m11 = sb("m11", [128, 1])
    v0 = sb("v0", [128, 1])
    v1 = sb("v1", [128, 1])
    rx = sb("rx", [128, 1])
    ry = sb("ry", [128, 1])

    A = mybir.AluOpType

    # --- semaphore bookkeeping (python-side counters) ---
    dma_n = [0]   # number of DMA completions so far (each adds 16)
    c_n = [0]     # compute milestones

    with nc.Block() as block:

        @block.sync
        def _(sync):
            # stage 0: consts + padded images
            sync.dma_start(out=tc[:, :], in_=tf4[0:1, :].to_broadcast([128, 6]))
            sync.dma_start(out=axs[:, :], in_=axg[:, :])
            sync.dma_start(out=ays[:, :], in_=ayg[0:1, :].to_broadcast([128, W]))
            # NOTE: tc loaded per-image below; this first load is img 0
            dma_n[0] += 3
            # zero tile (vector memsets it; wait)
            sync.wait_ge(s_c, 1)
            for b in range(B_PER):
                base = b * NP_IMG
                # interior rows 0..511, cols 0..511
                sync.dma_start(
                    out=bass.AP(imgp, base, [[PW, H], [1, W]]),
                    in_=img4[b, :, :],
                )
                # zero pad: cols 512..513 for rows 0..512 ; row 512 full
                sync.dma_start(
                    out=bass.AP(imgp, base + W, [[PW, PH], [1, 2]]),
                    in_=z[0:PH - 128 * (PH // 128), 0:2].to_broadcast([PH, 2])
                    if False
                    else bass.AP(z.tensor if hasattr(z, "tensor") else z, 0, [[0, PH], [1, 2]]),
                )
                sync.dma_start(
                    out=bass.AP(imgp, base + 512 * PW, [[0, 1], [1, PW]]),
                    in_=bass.AP(z.tensor if hasattr(z, "tensor") else z, 0, [[0, 1], [1, PW]]),
                )
                dma_n[0] += 3
            for _ in range(dma_n[0]):
                pass
            sync.then_inc_dummy = None

        # the above needs all DMAs tracked; simpler: sync waits handled below

    return None  # placeholder; real construction in _build()


# The class-based builder below is the real implementation.
class _Builder:
    pass


def kernel(Img, Tform):
    raise NotImplementedError("replaced below")


# revision 13
# speedup vs baseline: 27.9565x; 27.9565x over previous
"""Trainium2 Bass/Tile kernel for nn_Apply2DTform: batched affine warp with
round-nearest bilinear sampling.

Sharding: pure data parallel, batch 32 -> 8 cores x 4 images each.

Per image on device:
  - Build zero-padded image [513 x 514] in DRAM scratch.
  - Per 128-row output block: compute source coords x,y replicating the jax
    reference's exact f32 op order, round via the (2^23+2^22) magic-number
    trick (ties-to-even, same as jnp.round), clip, form bilinear weights.
  - Gather taps with gpsimd indirect DMA: one descriptor per (pixel,row-tap)
    fetching the f32 pair at (row, y0c:y0c+2).  Where clipping makes
    y1c == y0c the pair's odd element is wrong, so W01/W11 are folded into
    W00/W10 (odd element then has weight exactly 0).
  - Blend, DMA out.  The Tile framework schedules engines and inserts all
    semaphores/drains.

kernel(**inputs): full (32,512,512,1)+(32,6) in -> full (32,512,512,1) out.
"""
import os
import sys

sys.path.insert(0, "/opt/trn_rl_repo")

import numpy as np

import concourse.bass as bass
import concourse.mybir as mybir
import concourse.tile as tile
from concourse.bass_utils import run_bass_kernel_spmd

f32 = mybir.dt.float32
i32 = mybir.dt.int32
A = mybir.AluOpType

N_CORES = 8
B_PER = 4
H = W = 512
PH, PW = 513, 514
NP_IMG = PH * PW
MAGIC = 12582912.0  # 2^23 + 2^22

LAST_EXEC_NS = None
LAST_RESULTS = None
_LEGALIZE = True  # sim can't handle injected NoOps; set False for --sim


def _ax_host():
    i = np.arange(512, dtype=np.float32)
    delta = np.float32(2.0) / np.float32(511.0)
    return (i * delta + np.float32(-1.0)).astype(np.float32)


def _build():
    nc = bass.Bass()
    img4 = nc.declare_dram_parameter("img4", [B_PER, H, W], f32, isOutput=False)
    tf4 = nc.declare_dram_parameter("tf4", [1, B_PER * 6], f32, isOutput=False)
    axg = nc.declare_dram_parameter("axg", [128, 4], f32, isOutput=False)
    ayg = nc.declare_dram_parameter("ayg", [1, W], f32, isOutput=False)
    zc = nc.declare_dram_parameter("zc", [1, PW], f32, isOutput=False)
    out4 = nc.declare_dram_parameter("out4", [B_PER, H, W], f32, isOutput=True)
    imgp = nc.dram_tensor("imgp", [B_PER * NP_IMG, 1], f32)

    with tile.TileContext(nc) as tc:
        with (
            tc.tile_pool(name="cst", bufs=1) as cst,
            tc.tile_pool(name="mth", bufs=2) as mth,
            tc.tile_pool(name="gth", bufs=2) as gth,
        ):
            tcall = cst.tile([128, 24], f32)
            axs = cst.tile([128, 4], f32)
            ays = cst.tile([128, W], f32)
            nc.sync.dma_start(out=tcall[:, :], in_=tf4[0:1, :].to_broadcast([128, 24]))
            nc.sync.dma_start(out=axs[:, :], in_=axg[:, :])
            nc.sync.dma_start(out=ays[:, :], in_=ayg[0:1, :].to_broadcast([128, W]))
            for b in range(B_PER):
                base = b * NP_IMG
                nc.sync.dma_start(
                    out=bass.AP(imgp, base, [[PW, H], [1, W]]),
                    in_=img4[b, :, :],
                )
                nc.sync.dma_start(
                    out=bass.AP(imgp, base + W, [[PW, PH], [1, 2]]),
                    in_=bass.AP(zc, 0, [[0, PH], [1, 2]]),
                )
                nc.sync.dma_start(
                    out=bass.AP(imgp, base + 512 * PW, [[PW, 1], [1, PW]]),
                    in_=bass.AP(zc, 0, [[0, 1], [1, PW]]),
                )

            for k in range(B_PER * 4):
                b, r = divmod(k, 4)
                m00 = tcall[:, 6 * b + 0 : 6 * b + 1]
                m01 = tcall[:, 6 * b + 1 : 6 * b + 2]
                m10 = tcall[:, 6 * b + 2 : 6 * b + 3]
                m11 = tcall[:, 6 * b + 3 : 6 * b + 4]
                v0 = tcall[:, 6 * b + 4 : 6 * b + 5]
                v1 = tcall[:, 6 * b + 5 : 6 * b + 6]
                axr = axs[:, r : r + 1]

                rx = mth.tile([128, 1], f32, tag="rx")
                ry = mth.tile([128, 1], f32, tag="ry")
                x_t = mth.tile([128, W], f32, tag="x_t")
                y_t = mth.tile([128, W], f32, tag="y_t")
                x0 = mth.tile([128, W], f32, tag="x0")
                y0 = mth.tile([128, W], f32, tag="y0")
                x0c = mth.tile([128, W], f32, tag="x0c")
                x1c = mth.tile([128, W], f32, tag="x1c")
                y0c = mth.tile([128, W], f32, tag="y0c")
                y1c = mth.tile([128, W], f32, tag="y1c")
                fx1 = mth.tile([128, W], f32, tag="fx1")
                fx0 = mth.tile([128, W], f32, tag="fx0")
                fy1 = mth.tile([128, W], f32, tag="fy1")
                fy0 = mth.tile([128, W], f32, tag="fy0")
                w00 = mth.tile([128, W], f32, tag="w00")
                w01 = mth.tile([128, W], f32, tag="w01")
                w10 = mth.tile([128, W], f32, tag="w10")
                w11 = mth.tile([128, W], f32, tag="w11")
                eqt = mth.tile([128, W], f32, tag="eqt")
                tmp = mth.tile([128, W], f32, tag="tmp")
                tm2 = mth.tile([128, W], f32, tag="tm2")
                adA = mth.tile([128, W], f32, tag="adA")
                adB = mth.tile([128, W], f32, tag="adB")
                offa = gth.tile([128, W], i32, tag="offa")
                offb = gth.tile([128, W], i32, tag="offb")
                ga = gth.tile([128, 2 * W], f32, tag="ga")
                gb = gth.tile([128, 2 * W], f32, tag="gb")
                acc = gth.tile([128, W], f32, tag="acc")

                # x = ((M00*ax_i + M01*ay_j) + V0 -> *0.5+... exact ref order)
                nc.vector.tensor_scalar(out=rx[:, :], in0=axr, scalar1=m00,
                                        scalar2=None, op0=A.mult)
                nc.vector.tensor_scalar(out=x_t[:, :], in0=ays[:, :], scalar1=m01,
                                        scalar2=None, op0=A.mult)
                nc.vector.tensor_scalar(out=x_t[:, :], in0=x_t[:, :], scalar1=rx[:, :],
                                        scalar2=v0, op0=A.add, op1=A.add)
                nc.vector.tensor_scalar(out=x_t[:, :], in0=x_t[:, :], scalar1=1.0,
                                        scalar2=0.5, op0=A.add, op1=A.mult)
                nc.vector.tensor_scalar(out=x_t[:, :], in0=x_t[:, :], scalar1=511.0,
                                        scalar2=None, op0=A.mult)
                nc.vector.tensor_scalar(out=x0[:, :], in0=x_t[:, :], scalar1=MAGIC,
                                        scalar2=MAGIC, op0=A.add, op1=A.subtract)
                nc.vector.tensor_scalar(out=x0c[:, :], in0=x0[:, :], scalar1=0.0,
                                        scalar2=512.0, op0=A.max, op1=A.min)
                nc.vector.tensor_scalar(out=x1c[:, :], in0=x0[:, :], scalar1=-1.0,
                                        scalar2=1.0, op0=A.max, op1=A.add)
                nc.vector.tensor_scalar(out=x1c[:, :], in0=x1c[:, :], scalar1=512.0,
                                        scalar2=None, op0=A.min)
                nc.vector.tensor_scalar(out=ry[:, :], in0=axr, scalar1=m10,
                                        scalar2=None, op0=A.mult)
                nc.vector.tensor_scalar(out=y_t[:, :], in0=ays[:, :], scalar1=m11,
                                        scalar2=None, op0=A.mult)
                nc.vector.tensor_scalar(out=y_t[:, :], in0=y_t[:, :], scalar1=ry[:, :],
                                        scalar2=v1, op0=A.add, op1=A.add)
                nc.vector.tensor_scalar(out=y_t[:, :], in0=y_t[:, :], scalar1=1.0,
                                        scalar2=0.5, op0=A.add, op1=A.mult)
                nc.vector.tensor_scalar(out=y_t[:, :], in0=y_t[:, :], scalar1=511.0,
                                        scalar2=None, op0=A.mult)
                nc.vector.tensor_scalar(out=y0[:, :], in0=y_t[:, :], scalar1=MAGIC,
                                        scalar2=MAGIC, op0=A.add, op1=A.subtract)
                nc.vector.tensor_scalar(out=y0c[:, :], in0=y0[:, :], scalar1=0.0,
                                        scalar2=512.0, op0=A.max, op1=A.min)
                nc.vector.tensor_scalar(out=y1c[:, :], in0=y0[:, :], scalar1=-1.0,
                                        scalar2=1.0, op0=A.max, op1=A.add)
                nc.vector.tensor_scalar(out=y1c[:, :], in0=y1c[:, :], scalar1=512.0,
                                        scalar2=None, op0=A.min)
                # fractions / weights
                nc.vector.tensor_tensor(out=fx1[:, :], in0=x1c[:, :], in1=x_t[:, :],
                                        op=A.subtract)
                nc.vector.tensor_tensor(out=fx0[:, :], in0=x_t[:, :], in1=x0c[:, :],
                                        op=A.subtract)
                nc.vector.tensor_tensor(out=fy1[:, :], in0=y1c[:, :], in1=y_t[:, :],
                                        op=A.subtract)
                nc.vector.tensor_tensor(out=fy0[:, :], in0=y_t[:, :], in1=y0c[:, :],
                                        op=A.subtract)
                nc.vector.tensor_tensor(out=w00[:, :], in0=fx1[:, :], in1=fy1[:, :],
                                        op=A.mult)
                nc.vector.tensor_tensor(out=w01[:, :], in0=fx1[:, :], in1=fy0[:, :],
                                        op=A.mult)
                nc.vector.tensor_tensor(out=w10[:, :], in0=fx0[:, :], in1=fy1[:, :],
                                        op=A.mult)
                nc.vector.tensor_tensor(out=w11[:, :], in0=fx0[:, :], in1=fy0[:, :],
                                        op=A.mult)
                # fold odd-element weights where y1c == y0c (clip collision)
                nc.vector.tensor_tensor(out=eqt[:, :], in0=y0c[:, :], in1=y1c[:, :],
                                        op=A.is_equal)
                nc.vector.tensor_tensor(out=tmp[:, :], in0=eqt[:, :], in1=w01[:, :],
                                        op=A.mult)
                nc.vector.tensor_tensor(out=w00[:, :], in0=w00[:, :], in1=tmp[:, :],
                                        op=A.add)
                nc.vector.tensor_tensor(out=w01[:, :], in0=w01[:, :], in1=tmp[:, :],
                                        op=A.subtract)
                nc.vector.tensor_tensor(out=tm2[:, :], in0=eqt[:, :], in1=w11[:, :],
                                        op=A.mult)
                nc.vector.tensor_tensor(out=w10[:, :], in0=w10[:, :], in1=tm2[:, :],
                                        op=A.add)
                nc.vector.tensor_tensor(out=w11[:, :], in0=w11[:, :], in1=tm2[:, :],
                                        op=A.subtract)
                # flat addresses
                nc.vector.scalar_tensor_tensor(out=adA[:, :], in0=x0c[:, :],
                                               scalar=float(PW), in1=y0c[:, :],
                                               op0=A.mult, op1=A.add)
                nc.vector.scalar_tensor_tensor(out=adB[:, :], in0=x1c[:, :],
                                               scalar=float(PW), in1=y0c[:, :],
                                               op0=A.mult, op1=A.add)
                nc.vector.tensor_copy(out=offa[:, :], in_=adA[:, :])
                nc.vector.tensor_copy(out=offb[:, :], in_=adB[:, :])
                # gathers: 2 f32 per descriptor
                nc.gpsimd.indirect_dma_start(
                    out=ga[:, :], out_offset=None, in_=imgp[:, :],
                    in_offset=bass.IndirectOffsetOnAxis(ap=offa[:, :], axis=0),
                    element_offset=b * NP_IMG,
                )
                nc.gpsimd.indirect_dma_start(
                    out=gb[:, :], out_offset=None, in_=imgp[:, :],
                    in_offset=bass.IndirectOffsetOnAxis(ap=offb[:, :], axis=0),
                    element_offset=b * NP_IMG,
                )
                # blend
                ga_e = bass.AP(ga.tensor, ga[:, :].offset, [[2 * W, 128], [2, W]])
                ga_o = bass.AP(ga.tensor, ga[:, :].offset + 1, [[2 * W, 128], [2, W]])
                gb_e = bass.AP(gb.tensor, gb[:, :].offset, [[2 * W, 128], [2, W]])
                gb_o = bass.AP(gb.tensor, gb[:, :].offset + 1, [[2 * W, 128], [2, W]])
                nc.vector.tensor_tensor(out=acc[:, :], in0=w00[:, :], in1=ga_e,
                                        op=A.mult)
                nc.vector.tensor_tensor(out=tmp[:, :], in0=w01[:, :], in1=ga_o,
                                        op=A.mult)
                nc.vector.tensor_tensor(out=acc[:, :], in0=acc[:, :], in1=tmp[:, :],
                                        op=A.add)
                nc.vector.tensor_tensor(out=tm2[:, :], in0=w10[:, :], in1=gb_e,
                                        op=A.mult)
                nc.vector.tensor_tensor(out=acc[:, :], in0=acc[:, :], in1=tm2[:, :],
                                        op=A.add)
                nc.vector.tensor_tensor(out=tmp[:, :], in0=w11[:, :], in1=gb_o,
                                        op=A.mult)
                nc.vector.tensor_tensor(out=acc[:, :], in0=acc[:, :], in1=tmp[:, :],
                                        op=A.add)
                nc.sync.dma_start(out=out4[b, 128 * r : 128 * (r + 1), :],
                                  in_=acc[:, :])

    if _LEGALIZE:
        _legalize_multiwaits(nc)
    return nc


def _legalize_multiwaits(nc):
    """This container's walrus cannot encode >1 sem-wait on engine (non-DMA)
    instructions; split extras onto chained wait-NoOps on the same engine."""
    ctr = [0]

    def fresh(engine, wait):
        ctr[0] += 1
        n = mybir.InstNoOp(name=f"I-mwfix-{ctr[0]}", ins=[], outs=[])
        n.engine = engine
        n.sync_info = mybir.SyncInfo(on_wait=[wait], on_update=[])
        n.bass_nofuse = True
        return n

    for fn in nc.m.functions:
        for blk in fn.blocks:
            out = []
            changed = False
            for inst in blk.instructions:
                si = inst.sync_info
                if si is not None and len(si.on_wait) > 1:
                    waits = list(si.on_wait)
                    for w in waits[1:]:
                        out.append(fresh(inst.engine, w))
                    inst.sync_info = mybir.SyncInfo(
                        on_wait=[waits[0]], on_update=list(si.on_update)
                    )
                    changed = True
                out.append(inst)
            if changed:
                blk.instructions = out


_NC = None


def _get_nc():
    global _NC
    if _NC is None:
        _NC = _build()
    return _NC


def kernel(Img, Tform):
    global LAST_EXEC_NS, LAST_RESULTS
    Img = np.ascontiguousarray(np.asarray(Img, dtype=np.float32))
    Tform = np.ascontiguousarray(np.asarray(Tform, dtype=np.float32))
    assert Img.shape == (32, 512, 512, 1) and Tform.shape == (32, 6)

    nc = _get_nc()
    ax = _ax_host()
    axg = np.ascontiguousarray(ax.reshape(4, 128).T)
    ayg = ax.reshape(1, 512).copy()
    zcv = np.zeros((1, PW), dtype=np.float32)

    in_maps = []
    for k in range(N_CORES):
        sl = slice(B_PER * k, B_PER * (k + 1))
        in_maps.append({
            "img4": np.ascontiguousarray(Img[sl, :, :, 0]),
            "tf4": np.ascontiguousarray(Tform[sl].reshape(1, B_PER * 6)),
            "axg": axg,
            "ayg": ayg,
            "zc": zcv,
        })

    trace = bool(int(os.environ.get("WARP_TRACE", "0")))
    res = run_bass_kernel_spmd(nc, in_maps, list(range(N_CORES)), trace=trace)
    LAST_EXEC_NS = res.exec_time_ns
    LAST_RESULTS = res

    out = np.empty((32, 512, 512, 1), dtype=np.float32)
    for k in range(N_CORES):
        out[B_PER * k : B_PER * (k + 1), :, :, 0] = res.results[k]["out4"]
    return out
